# revision 12
# baseline (speedup 1.0000x reference)
"""AttentionalJoin kernel for 8 Trainium2 NeuronCores.

Math: the reference builds full (M x M) self-attention over M = N+1 tokens
(CLS prepended) but returns only the CLS row of the projected output.  Only
the CLS query survives, so attention collapses to a softmax-weighted token
pooling:

    q       = Wq @ cls                       (per head h: q_h)
    score_t = scale * q_h . (Wk x_t)_h  =  x_t . R[:, h],   R = scale*Wk_h^T q_h
    p       = softmax over the M tokens (scores bounded ~[-6, 6]; no max-sub)
    pooled_h = sum_t p_t x_t                 (linearity: project AFTER pooling)
    out     = proj( concat_h Wv_h pooled_h ) + proj_b

Device data flow (per 512-token chunk; x is passed BOTH natural and
host-pre-transposed, so TensorE never builds x^T):

    PE scores:  psc[8,512] = sum_q R_q^T @ xt_q      (4 matmuls, N=512)
    ACT exp:    e[8,512] = exp(psc), accum -> z
    PE etrans:  pet[128,8] = e_j^T @ I8              (4 tiny matmuls)
    PE pool:    ps[b][8,512] += pet_j^T @ x_j        (4 matmuls, N=512)

The PE streams X twice (scores rhs + pool rhs) at 1 col/cycle ~= 14us
instead of the 34us of the identity-transpose pipeline.  DMA layouts use
4KB-contiguous per-partition runs (token = a*512 + 4p + u) so descriptor
generation doesn't serialize the start; the host permutes x^T's columns to
match.  A junk-matmul warmup burst at the start keeps the PE busy while
the first chunk streams in, flipping the HAM clock gate (1.2 -> 2.4 GHz)
before the real matmuls begin.

The tiny tail (head-mix with Wv, proj, bias, cls-token contribution) runs
on host in fp32/fp64.

Sharding: data-parallel over the batch dim, 2 batches per core.
"""

import numpy as np

H = 8
C = 512
HD = C // H
B = 16
N = 2048
NCORES = 8
BPC = B // NCORES          # batches per core
TOK = BPC * N              # tokens per core (4096)
NCHUNK = TOK // 512        # 512-token compute chunks per core (8)
NWARM = 8                  # junk matmuls to pre-warm the HAM clock gate
MAX_DRAIN_WAITS = 1        # this walrus rejects instructions w/ >1 sem wait

_cached = {}


def _patch_drain():
    """The container's walrus codegen rejects instructions carrying more
    than one sem wait ("Too many sync wait commands").  Split extra waits
    onto dedicated same-engine NOPs, which preserves semantics (engine
    queues are in-order)."""
    import concourse.tile as tile_mod
    from concourse import mybir
    from bass_rust import ScopedClock

    if getattr(tile_mod.TileContext, "_drain_patched", False):
        return

    orig_lower = tile_mod.TileContext._lower_ordered_insts

    def _lower_ordered_insts(self, ordered):
        nc = self.nc
        for bbname, insts in ordered.items():
            out = []
            for inst in insts:
                si = inst.sync_info
                if si is not None and si.on_wait and len(si.on_wait) > MAX_DRAIN_WAITS:
                    waits = list(si.on_wait)
                    extra, keep = waits[:-MAX_DRAIN_WAITS], waits[-MAX_DRAIN_WAITS:]
                    for w in extra:
                        nop = mybir.InstNoOp(
                            name=f"waitsplit-{nc.next_id()}",
                            engine=inst.engine,
                            ins=[],
                            outs=[],
                            bass_nofuse=True,
                            sync_info=mybir.SyncInfo(on_wait=[w], on_update=[]),
                            debug=inst.debug,
                        )
                        out.append(nop)
                    inst.sync_info = mybir.SyncInfo(
                        on_wait=keep, on_update=list(si.on_update)
                    )
                out.append(inst)
            ordered[bbname] = out
        return orig_lower(self, ordered)

    tile_mod.TileContext._lower_ordered_insts = _lower_ordered_insts

    def _drain_and_barrier(self, tick_clock, wait_clock):
        nc = self.nc
        probe = mybir.InstNoOp(
            name=f"drain-wait-probe-{nc.next_id()}",
            engine=mybir.EngineType.SP,
            ins=[],
            outs=[],
        )
        wait_clock.add_sem_waits(probe, ScopedClock({None: tick_clock.global_clock}))
        waits = list(probe.sync_info.on_wait) if probe.sync_info else []
        for i in range(0, len(waits), MAX_DRAIN_WAITS):
            chunk = waits[i : i + MAX_DRAIN_WAITS]
            nop = nc.sync.nop(nofuse=True, hint="drain_wait")
            nop.ins.sync_info = mybir.SyncInfo(on_wait=chunk, on_update=[])
        nc.sync.drain()

        nc.all_engine_barrier()
        popped = nc._tile_sem_poison_stack.pop()
        assert popped is self._sem_poison
        nc.clear_and_free_semaphores(list(self.sems.allocated().values()))
        nc.all_engine_barrier()

    tile_mod.TileContext._drain_and_barrier = _drain_and_barrier
    tile_mod.TileContext._drain_patched = True


def _build_module():
    import concourse.bass as bass
    import concourse.tile as tile
    from concourse import mybir
    from concourse.masks import make_identity

    _patch_drain()
    f16 = mybir.dt.float16
    f32 = mybir.dt.float32
    EXP = mybir.ActivationFunctionType.Exp

    nc = bass.Bass()
    # xy packs, per chunk a and lane p, 8KB contiguous: 4 q-rows of x^T
    # (x^T[q*128+p, chunk-a tokens]) then 4 token-rows of natural x
    # (tokens a*512 + 4p + u).  One 128-descriptor DMA per chunk.
    xy_in = nc.dram_tensor("xy", [NCHUNK, 128, 8, 512], f16, kind="ExternalInput")
    r_in = nc.dram_tensor("r", [C, H], f16, kind="ExternalInput")
    s_out = nc.dram_tensor("s", [BPC, H, C], f32, kind="ExternalOutput")
    z_out = nc.dram_tensor("z", [BPC, H, N // 512], f32, kind="ExternalOutput")

    r_src = r_in.rearrange("(q p) h -> p q h", p=128)

    with tile.TileContext(nc) as tc:
        with (
            tc.tile_pool(name="xpool", bufs=1) as xpool,
            tc.tile_pool(name="consts", bufs=1) as consts,
            tc.tile_pool(name="epool", bufs=1) as epool,
            tc.tile_pool(name="opool", bufs=2) as opool,
            tc.tile_pool(name="psc", bufs=2, space="PSUM") as psc_pool,
            tc.tile_pool(name="pet", bufs=2, space="PSUM") as pet_pool,
            tc.tile_pool(name="pjunk", bufs=1, space="PSUM") as pjunk_pool,
            tc.tile_pool(name="ps", bufs=1, space="PSUM") as ps_pool,
        ):
            r_sb = consts.tile([128, 4, H], f16)
            nc.sync.dma_start(out=r_sb, in_=r_src)
            ident = consts.tile([128, 128], f16)
            make_identity(nc, ident)

            # HAM warmup: junk matmuls on the PE while x streams in.  The
            # rhs is a memset tile so values are defined; results go to a
            # scratch PSUM bank nobody reads.
            junk = consts.tile([128, 512], f16)
            nc.vector.memset(junk, 0.0)
            junk_ps = pjunk_pool.tile([H, 512], f32, tag="junkps", name="junkps")
            for w in range(NWARM):
                nc.tensor.matmul(
                    junk_ps, r_sb[:, 0, :], junk, start=True, stop=True
                )

            # alternate issue engines so descriptor generation for the 8
            # chunk DMAs runs on two sequencers in parallel
            xy_sb = []
            for a in range(NCHUNK):
                t = xpool.tile([128, 8, 512], f16, tag=f"xy{a}", name=f"xy{a}")
                eng = nc.sync if a % 2 == 0 else nc.scalar
                eng.dma_start(out=t, in_=xy_in[a])
                xy_sb.append(t)

            e_sb = [
                epool.tile([H, 512], f16, tag=f"e{a}", name=f"e{a}")
                for a in range(NCHUNK)
            ]
            et = [
                epool.tile([128, 4, H], f16, tag=f"et{a}", name=f"et{a}")
                for a in range(NCHUNK)
            ]
            zp = [
                epool.tile([H, N // 512], f32, tag=f"zp{b}", name=f"zp{b}")
                for b in range(BPC)
            ]
            ps = [
                ps_pool.tile([H, C], f32, tag=f"ps{b}", name=f"psacc{b}")
                for b in range(BPC)
            ]

            ncopy = 0

            def chunk(a):
                nonlocal ncopy
                b, g = divmod(a, 4)
                psc = psc_pool.tile([H, 512], f32, tag="psc", name=f"psc{a}")
                for q in range(4):
                    nc.tensor.matmul(
                        psc,
                        r_sb[:, q, :],
                        xy_sb[a][:, q, :],
                        start=(q == 0),
                        stop=(q == 3),
                    )
                nc.scalar.activation(
                    out=e_sb[a],
                    in_=psc,
                    func=EXP,
                    accum_out=zp[b][:, g : g + 1],
                )
                # transpose E via matmul: pet[128, 8] = e_slice^T @ I8;
                # free index of e / psc is u*128 + p -> token a*512 + 4p + u,
                # so pet_u's lane p matches x_sb[a][:, u, :] (token 4p + u)
                for u in range(4):
                    pet = pet_pool.tile([128, H], f32, tag="pet", name=f"pet{a}_{u}")
                    nc.tensor.matmul(
                        pet,
                        e_sb[a][:, u * 128 : (u + 1) * 128],
                        ident[:H, :H],
                        start=True,
                        stop=True,
                    )
                    if ncopy % 2 == 0:
                        nc.vector.tensor_copy(et[a][:, u, :], pet)
                    else:
                        nc.scalar.copy(et[a][:, u, :], pet)
                    ncopy += 1
                for u in range(4):
                    nc.tensor.matmul(
                        ps[b],
                        et[a][:, u, :],
                        xy_sb[a][:, 4 + u, :],
                        start=(g == 0 and u == 0),
                        stop=(g == 3 and u == 3),
                    )

            def emit_out(b):
                so = opool.tile([H, C], f32, tag=f"so{b}", name=f"so{b}")
                nc.vector.tensor_copy(so, ps[b])
                nc.gpsimd.dma_start(out=s_out[b], in_=so)
                nc.gpsimd.dma_start(out=z_out[b], in_=zp[b])

            for a in range(NCHUNK):
                chunk(a)
                if a == 3:
                    emit_out(0)
            emit_out(1)

    return nc


def _get_module():
    if "nc" not in _cached:
        _cached["nc"] = _build_module()
    return _cached["nc"]


def _host_prep(cls, qkv_w):
    scale = HD ** -0.5
    c = cls.reshape(C).astype(np.float64)
    Wq = qkv_w[:C].astype(np.float64)
    Wk = qkv_w[C : 2 * C].astype(np.float64)
    q = Wq @ c
    qh = q.reshape(H, HD)
    Wkh = Wk.reshape(H, HD, C)
    R = (scale * np.einsum("hdc,hd->ch", Wkh, qh)).astype(np.float16)
    k0 = Wk @ c
    score0 = scale * np.einsum("hd,hd->h", qh, k0.reshape(H, HD))
    e0 = np.exp(score0)
    return R, e0


def _make_in_maps(x16, R):
    """x16: [B*N, C] fp16.  Per core, pack one [NCHUNK, 128, 8, 512]
    tensor: slot g<4 holds x^T rows q*128+p for chunk a (token order
    u*128 + p_tok, i.e. token a*512 + 4*p_tok + u), slots g>=4 hold
    natural-x tokens a*512 + 4p + u."""
    maps = []
    for i in range(NCORES):
        xc = x16[i * TOK : (i + 1) * TOK].reshape(NCHUNK, 128, 4, C)  # [a,p,u,f]
        xy = np.empty((NCHUNK, 128, 8, 512), np.float16)
        # x^T part: xy[a, p, q, u*128 + pt] = xc[a, pt, u, q*128 + p]
        xt = xc.transpose(0, 3, 2, 1).reshape(NCHUNK, 4, 128, 4 * 128)  # [a,q,p,(u pt)]
        xy[:, :, 0:4, :] = xt.transpose(0, 2, 1, 3)
        xy[:, :, 4:8, :] = xc.transpose(0, 1, 2, 3)
        maps.append({"xy": np.ascontiguousarray(xy), "r": R})
    return maps


def kernel(x, cls, qkv_w, proj_w, proj_b):
    from concourse.bass_utils import run_bass_kernel_spmd

    x = np.asarray(x, dtype=np.float32)
    cls = np.asarray(cls, dtype=np.float32)
    qkv_w = np.asarray(qkv_w, dtype=np.float32)
    proj_w = np.asarray(proj_w, dtype=np.float32)
    proj_b = np.asarray(proj_b, dtype=np.float32)

    R, e0 = _host_prep(cls, qkv_w)
    Wv = qkv_w[2 * C :]

    x16 = np.ascontiguousarray(x.reshape(B * N, C).astype(np.float16))
    nc = _get_module()
    in_maps = _make_in_maps(x16, R)
    res = run_bass_kernel_spmd(nc, in_maps, list(range(NCORES)))
    _cached["last_results"] = res

    s_dev = np.concatenate([res.results[i]["s"] for i in range(NCORES)], axis=0)
    z_dev = np.concatenate(
        [res.results[i]["z"].sum(axis=-1) for i in range(NCORES)], axis=0
    )

    # add the CLS token's own contribution, normalize, head-mix + proj
    cf = cls.reshape(C)
    s_full = s_dev + (e0[:, None] * cf[None, :]).astype(np.float32)[None]
    z_full = z_dev + e0.astype(np.float32)[None]
    v = s_full / z_full[:, :, None]
    o = np.einsum("hdc,bhc->bhd", Wv.reshape(H, HD, C), v).reshape(B, C)
    y = o @ proj_w.T + proj_b
    return y.astype(np.float32)


# revision 16
# speedup vs baseline: 1.1295x; 1.1295x over previous
"""AttentionalJoin kernel for 8 Trainium2 NeuronCores.

Math: the reference builds full (M x M) self-attention over M = N+1 tokens
(CLS prepended) but returns only the CLS row of the projected output.  Only
the CLS query survives, so attention collapses to a softmax-weighted token
pooling:

    q       = Wq @ cls                       (per head h: q_h)
    score_t = scale * q_h . (Wk x_t)_h  =  x_t . R[:, h],   R = scale*Wk_h^T q_h
    p       = softmax over the M tokens (scores bounded ~[-6, 6]; no max-sub)
    pooled_h = sum_t p_t x_t                 (linearity: project AFTER pooling)
    out     = proj( concat_h Wv_h pooled_h ) + proj_b

Device data flow (per 512-token chunk; x is passed BOTH natural and
host-pre-transposed, so TensorE never builds x^T):

    PE scores:  psc[8,512] = sum_q R_q^T @ xt_q      (4 matmuls, N=512)
    ACT exp:    e[8,512] = exp(psc), accum -> z
    PE etrans:  pet[128,8] = e_j^T @ I8              (4 tiny matmuls)
    PE pool:    ps[b][8,512] += pet_j^T @ x_j        (4 matmuls, N=512)

The PE streams X twice (scores rhs + pool rhs) at 1 col/cycle ~= 14us
instead of the 34us of the identity-transpose pipeline.  DMA layouts use
4KB-contiguous per-partition runs (token = a*512 + 4p + u) so descriptor
generation doesn't serialize the start; the host permutes x^T's columns to
match.  A junk-matmul warmup burst at the start keeps the PE busy while
the first chunk streams in, flipping the HAM clock gate (1.2 -> 2.4 GHz)
before the real matmuls begin.

The tiny tail (head-mix with Wv, proj, bias, cls-token contribution) runs
on host in fp32/fp64.

Sharding: data-parallel over the batch dim, 2 batches per core.
"""

import numpy as np

H = 8
C = 512
HD = C // H
B = 16
N = 2048
NCORES = 8
BPC = B // NCORES          # batches per core
TOK = BPC * N              # tokens per core (4096)
NCHUNK = TOK // 512        # 512-token compute chunks per core (8)
NWARM = 8                  # junk matmuls to pre-warm the HAM clock gate
MAX_DRAIN_WAITS = 1        # this walrus rejects instructions w/ >1 sem wait

_cached = {}


def _patch_drain():
    """The container's walrus codegen rejects instructions carrying more
    than one sem wait ("Too many sync wait commands").  Split extra waits
    onto dedicated same-engine NOPs, which preserves semantics (engine
    queues are in-order)."""
    import concourse.tile as tile_mod
    from concourse import mybir
    from bass_rust import ScopedClock

    if getattr(tile_mod.TileContext, "_drain_patched", False):
        return

    orig_lower = tile_mod.TileContext._lower_ordered_insts

    def _lower_ordered_insts(self, ordered):
        nc = self.nc
        for bbname, insts in ordered.items():
            out = []
            for inst in insts:
                si = inst.sync_info
                if si is not None and si.on_wait and len(si.on_wait) > MAX_DRAIN_WAITS:
                    waits = list(si.on_wait)
                    extra, keep = waits[:-MAX_DRAIN_WAITS], waits[-MAX_DRAIN_WAITS:]
                    for w in extra:
                        nop = mybir.InstNoOp(
                            name=f"waitsplit-{nc.next_id()}",
                            engine=inst.engine,
                            ins=[],
                            outs=[],
                            bass_nofuse=True,
                            sync_info=mybir.SyncInfo(on_wait=[w], on_update=[]),
                            debug=inst.debug,
                        )
                        out.append(nop)
                    inst.sync_info = mybir.SyncInfo(
                        on_wait=keep, on_update=list(si.on_update)
                    )
                out.append(inst)
            ordered[bbname] = out
        return orig_lower(self, ordered)

    tile_mod.TileContext._lower_ordered_insts = _lower_ordered_insts

    def _drain_and_barrier(self, tick_clock, wait_clock):
        nc = self.nc
        probe = mybir.InstNoOp(
            name=f"drain-wait-probe-{nc.next_id()}",
            engine=mybir.EngineType.SP,
            ins=[],
            outs=[],
        )
        wait_clock.add_sem_waits(probe, ScopedClock({None: tick_clock.global_clock}))
        waits = list(probe.sync_info.on_wait) if probe.sync_info else []
        for i in range(0, len(waits), MAX_DRAIN_WAITS):
            chunk = waits[i : i + MAX_DRAIN_WAITS]
            nop = nc.sync.nop(nofuse=True, hint="drain_wait")
            nop.ins.sync_info = mybir.SyncInfo(on_wait=chunk, on_update=[])
        nc.sync.drain()

        nc.all_engine_barrier()
        popped = nc._tile_sem_poison_stack.pop()
        assert popped is self._sem_poison
        nc.clear_and_free_semaphores(list(self.sems.allocated().values()))
        nc.all_engine_barrier()

    tile_mod.TileContext._drain_and_barrier = _drain_and_barrier
    tile_mod.TileContext._drain_patched = True


def _build_module():
    import concourse.bass as bass
    import concourse.tile as tile
    from concourse import mybir
    from concourse.masks import make_identity

    _patch_drain()
    f16 = mybir.dt.float16
    f32 = mybir.dt.float32
    EXP = mybir.ActivationFunctionType.Exp

    nc = bass.Bass()
    # xy packs, per chunk a and lane p, 8KB contiguous: 4 q-rows of x^T
    # (x^T[q*128+p, chunk-a tokens]) then 4 token-rows of natural x
    # (tokens a*512 + 4p + u).  One 128-descriptor DMA per chunk.
    xy_in = nc.dram_tensor("xy", [NCHUNK, 128, 8, 512], f16, kind="ExternalInput")
    r_in = nc.dram_tensor("r", [C, H], f16, kind="ExternalInput")
    s_out = nc.dram_tensor("s", [BPC, H, C], f32, kind="ExternalOutput")
    z_out = nc.dram_tensor("z", [BPC, H, N // 512], f32, kind="ExternalOutput")

    r_src = r_in.rearrange("(q p) h -> p q h", p=128)

    with tile.TileContext(nc) as tc:
        with (
            tc.tile_pool(name="xpool", bufs=1) as xpool,
            tc.tile_pool(name="consts", bufs=1) as consts,
            tc.tile_pool(name="epool", bufs=1) as epool,
            tc.tile_pool(name="opool", bufs=2) as opool,
            tc.tile_pool(name="psc", bufs=2, space="PSUM") as psc_pool,
            tc.tile_pool(name="pet", bufs=2, space="PSUM") as pet_pool,
            tc.tile_pool(name="pjunk", bufs=1, space="PSUM") as pjunk_pool,
            tc.tile_pool(name="ps", bufs=1, space="PSUM") as ps_pool,
        ):
            r_sb = consts.tile([128, 4, H], f16)
            nc.sync.dma_start(out=r_sb, in_=r_src)
            ident = consts.tile([128, 128], f16)
            make_identity(nc, ident)

            # HAM warmup: junk matmuls on the PE while x streams in.  The
            # rhs is a memset tile so values are defined; results go to a
            # scratch PSUM bank nobody reads.
            junk = consts.tile([128, 512], f16)
            nc.vector.memset(junk, 0.0)
            junk_ps = pjunk_pool.tile([H, 512], f32, tag="junkps", name="junkps")
            for w in range(NWARM):
                nc.tensor.matmul(
                    junk_ps, r_sb[:, 0, :], junk, start=True, stop=True
                )

            # two DMA instructions per chunk for finer completion pacing
            xy_sb = []
            for a in range(NCHUNK):
                t = xpool.tile([128, 8, 512], f16, tag=f"xy{a}", name=f"xy{a}")
                nc.sync.dma_start(out=t[:, 0:4, :], in_=xy_in[a, :, 0:4, :])
                nc.sync.dma_start(out=t[:, 4:8, :], in_=xy_in[a, :, 4:8, :])
                xy_sb.append(t)

            e_sb = [
                epool.tile([H, 512], f16, tag=f"e{a}", name=f"e{a}")
                for a in range(NCHUNK)
            ]
            et = [
                epool.tile([128, 4, H], f16, tag=f"et{a}", name=f"et{a}")
                for a in range(NCHUNK)
            ]
            zp = [
                epool.tile([H, N // 512], f32, tag=f"zp{b}", name=f"zp{b}")
                for b in range(BPC)
            ]
            ps = [
                ps_pool.tile([H, C], f32, tag=f"ps{b}", name=f"psacc{b}")
                for b in range(BPC)
            ]

            ncopy = 0

            def chunk(a):
                nonlocal ncopy
                b, g = divmod(a, 4)
                psc = psc_pool.tile([H, 512], f32, tag="psc", name=f"psc{a}")
                for q in range(4):
                    nc.tensor.matmul(
                        psc,
                        r_sb[:, q, :],
                        xy_sb[a][:, q, :],
                        start=(q == 0),
                        stop=(q == 3),
                    )
                nc.scalar.activation(
                    out=e_sb[a],
                    in_=psc,
                    func=EXP,
                    accum_out=zp[b][:, g : g + 1],
                )
                # transpose E via matmul: pet[128, 8] = e_slice^T @ I8;
                # free index of e / psc is u*128 + p -> token a*512 + 4p + u,
                # so pet_u's lane p matches x_sb[a][:, u, :] (token 4p + u)
                for u in range(4):
                    pet = pet_pool.tile([128, H], f32, tag="pet", name=f"pet{a}_{u}")
                    nc.tensor.matmul(
                        pet,
                        e_sb[a][:, u * 128 : (u + 1) * 128],
                        ident[:H, :H],
                        start=True,
                        stop=True,
                    )
                    if ncopy % 2 == 0:
                        nc.vector.tensor_copy(et[a][:, u, :], pet)
                    else:
                        nc.scalar.copy(et[a][:, u, :], pet)
                    ncopy += 1
                for u in range(4):
                    nc.tensor.matmul(
                        ps[b],
                        et[a][:, u, :],
                        xy_sb[a][:, 4 + u, :],
                        start=(g == 0 and u == 0),
                        stop=(g == 3 and u == 3),
                    )

            def emit_out(b):
                nc.gpsimd.dma_start(out=z_out[b], in_=zp[b])
                so = opool.tile([H, C], f32, tag=f"so{b}", name=f"so{b}")
                nc.vector.tensor_copy(so, ps[b])
                nc.gpsimd.dma_start(out=s_out[b], in_=so)

            for a in range(NCHUNK):
                chunk(a)
                if a == 3:
                    emit_out(0)
            emit_out(1)

    return nc


def _get_module():
    if "nc" not in _cached:
        _cached["nc"] = _build_module()
    return _cached["nc"]


def _host_prep(cls, qkv_w):
    scale = HD ** -0.5
    c = cls.reshape(C).astype(np.float64)
    Wq = qkv_w[:C].astype(np.float64)
    Wk = qkv_w[C : 2 * C].astype(np.float64)
    q = Wq @ c
    qh = q.reshape(H, HD)
    Wkh = Wk.reshape(H, HD, C)
    R = (scale * np.einsum("hdc,hd->ch", Wkh, qh)).astype(np.float16)
    k0 = Wk @ c
    score0 = scale * np.einsum("hd,hd->h", qh, k0.reshape(H, HD))
    e0 = np.exp(score0)
    return R, e0


def _make_in_maps(x16, R):
    """x16: [B*N, C] fp16.  Per core, pack one [NCHUNK, 128, 8, 512]
    tensor: slot g<4 holds x^T rows q*128+p for chunk a (token order
    u*128 + p_tok, i.e. token a*512 + 4*p_tok + u), slots g>=4 hold
    natural-x tokens a*512 + 4p + u."""
    maps = []
    for i in range(NCORES):
        xc = x16[i * TOK : (i + 1) * TOK].reshape(NCHUNK, 128, 4, C)  # [a,p,u,f]
        xy = np.empty((NCHUNK, 128, 8, 512), np.float16)
        # x^T part: xy[a, p, q, u*128 + pt] = xc[a, pt, u, q*128 + p]
        xt = xc.transpose(0, 3, 2, 1).reshape(NCHUNK, 4, 128, 4 * 128)  # [a,q,p,(u pt)]
        xy[:, :, 0:4, :] = xt.transpose(0, 2, 1, 3)
        xy[:, :, 4:8, :] = xc.transpose(0, 1, 2, 3)
        maps.append({"xy": np.ascontiguousarray(xy), "r": R})
    return maps


def kernel(x, cls, qkv_w, proj_w, proj_b):
    from concourse.bass_utils import run_bass_kernel_spmd

    x = np.asarray(x, dtype=np.float32)
    cls = np.asarray(cls, dtype=np.float32)
    qkv_w = np.asarray(qkv_w, dtype=np.float32)
    proj_w = np.asarray(proj_w, dtype=np.float32)
    proj_b = np.asarray(proj_b, dtype=np.float32)

    R, e0 = _host_prep(cls, qkv_w)
    Wv = qkv_w[2 * C :]

    x16 = np.ascontiguousarray(x.reshape(B * N, C).astype(np.float16))
    nc = _get_module()
    in_maps = _make_in_maps(x16, R)
    res = run_bass_kernel_spmd(nc, in_maps, list(range(NCORES)))
    _cached["last_results"] = res

    s_dev = np.concatenate([res.results[i]["s"] for i in range(NCORES)], axis=0)
    z_dev = np.concatenate(
        [res.results[i]["z"].sum(axis=-1) for i in range(NCORES)], axis=0
    )

    # add the CLS token's own contribution, normalize, head-mix + proj
    cf = cls.reshape(C)
    s_full = s_dev + (e0[:, None] * cf[None, :]).astype(np.float32)[None]
    z_full = z_dev + e0.astype(np.float32)[None]
    v = s_full / z_full[:, :, None]
    o = np.einsum("hdc,bhc->bhd", Wv.reshape(H, HD, C), v).reshape(B, C)
    y = o @ proj_w.T + proj_b
    return y.astype(np.float32)
